# revision 1
# baseline (speedup 1.0000x reference)
"""Trainium2 Bass kernel: per-batch segment-mean pooling + 3-layer MLP.

Reference computation (B=64, T=512, H=768, S=128):
  pooled[b,s,:] = mean over t of hidden[b,t,:] where statements_ids[b,t]==s
  x = gelu(pooled @ w1 + b1); x = gelu(x @ w2 + b2)
  out[b,s] = sigmoid(x @ w3 + b3)

Distribution: data-parallel over batch across 8 NeuronCores (8 batches per
core); MLP weights replicated.

Per-core algorithm (all matmuls on PE at fp32r 1 cycle/row):
  - Build the one-hot matrix MT[t,s] = (sid[t]==s) on DVE via
    tensor_scalar(is_equal) against an iota constant.
  - counts = MT.T @ ones            (PE)        -> inv = 1/max(counts,1) (DVE)
  - pooled_sums = MT.T @ hidden[b]  (PE, [S,H]) -> pooled = sums*inv     (DVE)
  - X^T tiles via PE transpose (pooled is [S,H] but the MLP wants [H, rows])
  - MLP batched over all 8 local batches: rows = 8*128 = 1024 moving dim,
    weights stationary; gelu/sigmoid + bias fused on ACT.
"""

import os
import sys

sys.path.insert(0, "/opt/trn_rl_repo")

import numpy as np

import concourse.bass as bass
import concourse.mybir as mybir
import concourse.tile as tile
from concourse import bacc, bass_utils

B, T, H, S = 64, 512, 768, 128
N_CORES = 8
BL = B // N_CORES  # local batches per core
P = 128
KT = T // P        # t-tiles per batch
KH = H // P        # h-tiles
R = BL * S         # MLP rows per core
RC = 2 * S         # moving-dim chunk (2 batches) -- >=256 keeps fp32r at 1 cyc/row
NRC = R // RC
HF = H + 2         # hidden padded with 2 constant 1.0 columns (counts trick)
CR_COLS = 134      # f32r packed consts (matmul operands): ident | w3
CF_COLS = 173      # f32 packed consts: iota | sid-bits | b1 | b2 | b3

_CACHE: dict = {}


def _build_program(act_func=None):
    f32, f32r, i32 = mybir.dt.float32, mybir.dt.float32r, mybir.dt.int32
    FT = mybir.ActivationFunctionType
    OP = mybir.AluOpType

    nc = bacc.Bacc("TRN2", target_bir_lowering=False, debug=False)
    hid = nc.dram_tensor("hidden", [BL, T, HF], f32r, kind="ExternalInput").ap()
    w1 = nc.dram_tensor("w1", [H, H], f32r, kind="ExternalInput").ap()
    w2 = nc.dram_tensor("w2", [H, H], f32r, kind="ExternalInput").ap()
    cpack_r = nc.dram_tensor("cpack_r", [P, CR_COLS], f32r, kind="ExternalInput").ap()
    cpack_f = nc.dram_tensor("cpack_f", [P, CF_COLS], f32, kind="ExternalInput").ap()
    out = nc.dram_tensor("out", [BL, S], f32, kind="ExternalOutput").ap()

    with tile.TileContext(nc) as tc:
        with (
            tc.tile_pool(name="consts", bufs=1) as consts,
            tc.tile_pool(name="wpool", bufs=1) as wpool,
            tc.tile_pool(name="hpool", bufs=1) as hpool,
            tc.tile_pool(name="mtpool", bufs=8) as mtpool,
            tc.tile_pool(name="small", bufs=3) as small,
            tc.tile_pool(name="xtpool", bufs=1) as xtpool,
            tc.tile_pool(name="ypool", bufs=1) as ypool,
            tc.tile_pool(name="ps", bufs=8, space="PSUM") as ps,
        ):
            # ---- all small constants arrive in ONE packed DMA (single
            # 1.2KB line per partition) so the hidden stream starts at once ----
            cpf_sb = consts.tile([P, CF_COLS], f32)
            nc.sync.dma_start(cpf_sb, cpack_f)
            cpr_sb = consts.tile([P, CR_COLS], f32r)
            nc.sync.dma_start(cpr_sb, cpack_r)
            ident_sb = cpr_sb[:, 0:P]
            w3_sb = cpr_sb[:, P : P + KH]
            iota_sb = cpf_sb[:, 0:P]
            sid_sb = cpf_sb[:, P : P + BL * KT].bitcast(i32)
            b1_sb = cpf_sb[:, 160:166]
            b2_sb = cpf_sb[:, 166:172]
            b3_sb = cpf_sb[0:1, 172:173]

            # ---- hidden + weight streaming on sync/HWDGE, ordered to match
            # the compute pipeline: hidden batches pace the pooling; weight
            # k-tiles trickle between batches so fc1/fc2 unlock per-k ----
            hbs = [None] * BL
            w1ks = [None] * KH
            w2ks = [None] * KH

            def load_hb(b):
                if b < 2:
                    # first two batches arrive per k-chunk so pooling starts
                    # on the first 0.4 MB instead of the full 1.6 MB batch
                    tiles = []
                    for k in range(KT):
                        t = hpool.tile([P, HF], f32r, tag=f"hb{b}k{k}", name=f"hb{b}k{k}")
                        nc.sync.dma_start(t, hid[b, k * P : (k + 1) * P, :])
                        tiles.append(t)
                    hbs[b] = tiles
                else:
                    hb = hpool.tile(
                        [P, KT, HF], f32r, tag=f"hb{2 + (b - 2) % 3}", name=f"hb{b}"
                    )
                    nc.sync.dma_start(hb, hid[b].rearrange("(k p) h -> p k h", p=P))
                    hbs[b] = hb

            def hb_slice(b, k, lo, hi):
                if b < 2:
                    return hbs[b][k][:, lo:hi]
                return hbs[b][:, k, lo:hi]

            def load_w(ws, wdram, k, nm):
                ws[k] = wpool.tile([P, H], f32r, tag=f"{nm}{k}", name=f"{nm}{k}")
                nc.sync.dma_start(ws[k], wdram[k * P : (k + 1) * P, :])

            load_hb(0)
            for k in range(3):
                load_w(w1ks, w1, k, "w1k")
            load_hb(1)
            for k in range(3, KH):
                load_w(w1ks, w1, k, "w1k")
            load_hb(2)
            load_hb(3)
            for k in range(KH):
                load_w(w2ks, w2, k, "w2k")
            load_hb(4)
            load_hb(5)
            load_hb(6)
            load_hb(7)

            xts = [xtpool.tile([P, R], f32r, tag=f"xt{k}", name=f"xt{k}") for k in range(KH)]
            y1s = [ypool.tile([P, R], f32r, tag=f"y1_{m}", name=f"y1_{m}") for m in range(KH)]
            y2s = [ypool.tile([P, R], f32r, tag=f"y2_{m}", name=f"y2_{m}") for m in range(KH)]
            pred = ypool.tile([1, R], f32, tag="pred")

            C0 = 512          # pooling psum chunk 0: cols [0, 512)
            C1 = HF - C0      # chunk 1: cols [512, 770) -- col 768 = counts

            pooleds = [None] * BL

            def pool_mm(b):
                sidf = small.tile([P, KT], f32, tag="sidf")
                nc.vector.tensor_copy(sidf, sid_sb[:, b * KT : (b + 1) * KT])
                mts = []
                for k in range(KT):
                    mt = mtpool.tile([P, P], f32r, tag="mt")
                    nc.vector.tensor_tensor(
                        mt,
                        iota_sb,
                        sidf[:, k : k + 1].to_broadcast((P, P)),
                        OP.is_equal,
                    )
                    mts.append(mt)
                # counts chunk first so the inv chain runs while pp0 matmuls
                pp1 = ps.tile([P, C1], f32, tag="ps")
                pp0 = ps.tile([P, C0], f32, tag="ps")
                # interleave the two accumulation groups per k-chunk: both
                # matmuls of an arrived chunk fire at once instead of pp1(k3)
                # blocking ready pp0 work in the in-order PE stream
                for k in range(KT):
                    nc.tensor.matmul(
                        pp1, lhsT=mts[k], rhs=hb_slice(b, k, C0, HF),
                        start=(k == 0), stop=(k == KT - 1),
                    )
                    nc.tensor.matmul(
                        pp0, lhsT=mts[k], rhs=hb_slice(b, k, 0, C0),
                        start=(k == 0), stop=(k == KT - 1),
                    )
                inv = small.tile([P, 1], f32, tag="inv")
                nc.vector.tensor_scalar(inv, pp1[:, H - C0 : H - C0 + 1], 1.0, None, OP.max)
                nc.vector.reciprocal(inv, inv)
                pooled = small.tile([P, H], f32r, tag="pooled")
                # normalize in transpose-consumption order, smallest first:
                # [0:128] unblocks transpose m0 immediately, [128:512] covers
                # m1-m3 while m0 runs, [512:768] covers m4-m5
                nc.vector.tensor_tensor(
                    pooled[:, 0:P], pp0[:, 0:P], inv[:, 0:1].to_broadcast((P, P)),
                    OP.mult,
                )
                nc.vector.tensor_tensor(
                    pooled[:, P:C0], pp0[:, P:C0],
                    inv[:, 0:1].to_broadcast((P, C0 - P)), OP.mult,
                )
                nc.vector.tensor_tensor(
                    pooled[:, C0:H], pp1[:, 0 : H - C0],
                    inv[:, 0:1].to_broadcast((P, H - C0)), OP.mult,
                )
                pooleds[b] = pooled

            def pool_tr(b):
                pooled = pooleds[b]
                for m in range(KH):
                    trp = ps.tile([P, P], f32r, tag="ps")
                    nc.tensor.transpose(trp, pooled[:, m * P : (m + 1) * P], ident_sb)
                    nc.vector.tensor_copy(xts[m][:, b * S : (b + 1) * S], trp)

            def fc(wks, b_sb, xs, outs, rc, func):
                for m in range(KH):
                    pt = ps.tile([P, RC], f32, tag="ps")
                    for k in range(KH):
                        nc.tensor.matmul(
                            pt,
                            lhsT=wks[k][:, m * P : (m + 1) * P],
                            rhs=xs[k][:, rc * RC : (rc + 1) * RC],
                            start=(k == 0),
                            stop=(k == KH - 1),
                        )
                    nc.scalar.activation(
                        outs[m][:, rc * RC : (rc + 1) * RC],
                        pt,
                        func,
                        bias=b_sb[:, m : m + 1],
                    )

            def fc3(rc):
                pt = ps.tile([1, RC], f32, tag="ps")
                for k in range(KH):
                    nc.tensor.matmul(
                        pt,
                        lhsT=w3_sb[:, k : k + 1],
                        rhs=y2s[k][:, rc * RC : (rc + 1) * RC],
                        start=(k == 0),
                        stop=(k == KH - 1),
                    )
                nc.scalar.activation(
                    pred[:, rc * RC : (rc + 1) * RC],
                    pt,
                    mybir.ActivationFunctionType.Sigmoid,
                    bias=b3_sb,
                )
                # stream this chunk's predictions out immediately; only the
                # final 1 KB remains on the critical path after the last sigmoid
                nc.sync.dma_start(
                    out.rearrange("b s -> (b s)")[rc * RC : (rc + 1) * RC],
                    pred[:, rc * RC : (rc + 1) * RC],
                )

            FT = mybir.ActivationFunctionType
            gelu = FT.Gelu if act_func is None else act_func
            pool_mm(0)
            pool_tr(0)
            pool_mm(1)
            pool_tr(1)
            fc(w1ks, b1_sb, xts, y1s, 0, gelu)
            pool_mm(2)
            pool_tr(2)
            pool_mm(3)
            pool_tr(3)
            fc(w1ks, b1_sb, xts, y1s, 1, gelu)
            fc(w2ks, b2_sb, y1s, y2s, 0, gelu)
            fc3(0)
            pool_mm(4)
            pool_tr(4)
            pool_mm(5)
            pool_tr(5)
            fc(w1ks, b1_sb, xts, y1s, 2, gelu)
            fc(w2ks, b2_sb, y1s, y2s, 1, gelu)
            fc3(1)
            pool_mm(6)
            pool_tr(6)
            pool_mm(7)
            pool_tr(7)
            fc(w1ks, b1_sb, xts, y1s, 3, gelu)
            fc(w2ks, b2_sb, y1s, y2s, 2, gelu)
            fc3(2)
            fc(w2ks, b2_sb, y1s, y2s, 3, gelu)
            fc3(3)

    nc.compile()
    return nc


def _get_program():
    if "nc" not in _CACHE:
        _CACHE["nc"] = _build_program()
    return _CACHE["nc"]


def _cpack(sid_shard, b1, b2, b3, w3):
    """Pack per-core constants into two tensors: f32r (matmul operands,
    the DMA may round these) and plain f32 (bit-exact: iota, sid bits,
    biases)."""
    cr = np.zeros((P, CR_COLS), dtype=np.float32)
    cr[:, 0:P] = np.eye(P, dtype=np.float32)
    cr[:, P : P + KH] = np.asarray(w3, np.float32).reshape(KH, P, 1)[:, :, 0].T
    cf = np.zeros((P, CF_COLS), dtype=np.float32)
    cf[:, 0:P] = np.arange(P, dtype=np.float32)[None, :]
    sid_cols = np.transpose(
        sid_shard.astype(np.int32).reshape(BL, KT, P), (2, 0, 1)
    ).reshape(P, BL * KT)
    cf[:, P : P + BL * KT] = sid_cols.view(np.float32)
    cf[:, 160:166] = np.asarray(b1, np.float32).reshape(KH, P).T
    cf[:, 166:172] = np.asarray(b2, np.float32).reshape(KH, P).T
    cf[0, 172] = np.float32(np.asarray(b3).reshape(-1)[0])
    return cr, cf


def make_in_maps(hidden, statements_ids, w1, b1, w2, b2, w3, b3):
    hidden = np.asarray(hidden, dtype=np.float32)
    pad = np.ones((*hidden.shape[:2], HF - H), dtype=np.float32)
    hidden = np.ascontiguousarray(np.concatenate([hidden, pad], axis=-1))
    sid = np.asarray(statements_ids, dtype=np.int32)
    w1 = np.ascontiguousarray(np.asarray(w1, dtype=np.float32))
    w2 = np.ascontiguousarray(np.asarray(w2, dtype=np.float32))
    in_maps = []
    for c in range(N_CORES):
        cr, cf = _cpack(sid[c * BL : (c + 1) * BL], b1, b2, b3, w3)
        in_maps.append(
            {
                "hidden": hidden[c * BL : (c + 1) * BL],
                "w1": w1,
                "w2": w2,
                "cpack_r": cr,
                "cpack_f": cf,
            }
        )
    return in_maps


def kernel(hidden, statements_ids, w1, b1, w2, b2, w3, b3, **kwargs):
    nc = _get_program()
    in_maps = make_in_maps(hidden, statements_ids, w1, b1, w2, b2, w3, b3)
    trace = bool(int(os.environ.get("KERNEL_TRACE", "0")))
    res = bass_utils.run_bass_kernel_spmd(
        nc, in_maps, core_ids=list(range(N_CORES)), trace=trace
    )
    _CACHE["last_results"] = res
    out = np.concatenate([res.results[c]["out"] for c in range(N_CORES)], axis=0)
    return out.astype(np.float32)



# revision 3
# speedup vs baseline: 1.2554x; 1.2554x over previous
"""Trainium2 Bass kernel: per-batch segment-mean pooling + 3-layer MLP.

Reference computation (B=64, T=512, H=768, S=128):
  pooled[b,s,:] = mean over t of hidden[b,t,:] where statements_ids[b,t]==s
  x = gelu(pooled @ w1 + b1); x = gelu(x @ w2 + b2)
  out[b,s] = sigmoid(x @ w3 + b3)

Distribution: data-parallel over batch across 8 NeuronCores (8 batches per
core); MLP weights replicated.

Per-core algorithm:
  - Host precomputes the count-normalized one-hot MT[t,s] = inv[s]*(sid[t]==s)
    (counts depend only on statements_ids) and ships it in fp16, along with
    fp16 hidden and weights.  fp16 keeps the PE at 1 cycle/row for any moving
    size and halves HBM traffic vs fp32; 10 mantissa bits keep the rel err
    ~1e-3 (tolerance 2e-2).
  - pooled^T tiles directly: matmul(lhsT=hidden[t,h-tile], rhs=MT[t,s])
    -> psum [128h, 128s], accumulated over the 4 t-tiles.  No PE transposes,
    no on-device one-hot build, no normalization chain: the psum already
    holds mean-pooled values in the [h, s] orientation the MLP wants.
  - psum -> SBUF copies (fp16) alternate between DVE and Pool engines.
  - MLP batched over all 8 local batches: rows = 8*128 = 1024 moving dim,
    weights stationary; gelu/sigmoid + bias fused on ACT.
"""

import os
import sys

sys.path.insert(0, "/opt/trn_rl_repo")

import numpy as np

import concourse.bass as bass
import concourse.mybir as mybir
import concourse.tile as tile
from concourse import bacc, bass_utils

B, T, H, S = 64, 512, 768, 128
N_CORES = 8
BL = B // N_CORES  # local batches per core
P = 128
KT = T // P        # t-tiles per batch
KH = H // P        # h-tiles
R = BL * S         # MLP rows per core
RC = 2 * S         # moving-dim chunk (2 batches)
NRC = R // RC

_CACHE: dict = {}


def _build_program():
    f32, f16 = mybir.dt.float32, mybir.dt.float16
    FT = mybir.ActivationFunctionType

    nc = bacc.Bacc("TRN2", target_bir_lowering=False, debug=False)
    hid = nc.dram_tensor("hidden", [BL, P, KT * H], f16, kind="ExternalInput").ap()
    mtp = nc.dram_tensor("mtp", [BL, P, KT * S], f16, kind="ExternalInput").ap()
    w1 = nc.dram_tensor("w1", [KH, P, H], f16, kind="ExternalInput").ap()
    w2 = nc.dram_tensor("w2", [KH, P, H], f16, kind="ExternalInput").ap()
    wpack = nc.dram_tensor("wpack", [P, KH], f16, kind="ExternalInput").ap()
    cpack = nc.dram_tensor("cpack", [P, 13], f32, kind="ExternalInput").ap()
    out = nc.dram_tensor("out", [BL, S], f32, kind="ExternalOutput").ap()

    with tile.TileContext(nc) as tc:
        with (
            tc.tile_pool(name="consts", bufs=1) as consts,
            tc.tile_pool(name="wpool", bufs=1) as wpool,
            tc.tile_pool(name="hpool", bufs=1) as hpool,
            tc.tile_pool(name="xtpool", bufs=1) as xtpool,
            tc.tile_pool(name="ypool", bufs=1) as ypool,
            tc.tile_pool(name="ps", bufs=8, space="PSUM") as ps,
        ):
            cpf_sb = consts.tile([P, 13], f32)
            nc.sync.dma_start(cpf_sb, cpack)
            w3_sb = consts.tile([P, KH], f16, name="w3_sb")
            nc.sync.dma_start(w3_sb, wpack)
            b1_sb = cpf_sb[:, 0:KH]
            b2_sb = cpf_sb[:, KH : 2 * KH]
            b3_sb = cpf_sb[0:1, 12:13]

            mts = [None] * BL
            hbs = [None] * BL
            w1ks = [None] * KH
            w2ks = [None] * KH

            def load_mt(b):
                mts[b] = hpool.tile([P, KT * S], f16, tag=f"mt{b}", name=f"mt{b}")
                nc.sync.dma_start(mts[b], mtp[b])

            def load_hb(b):
                if b < 2:
                    # first two batches arrive per t-tile so pooling starts
                    # on the first 0.2 MB instead of the full 0.8 MB batch
                    tiles = []
                    for k in range(KT):
                        t = hpool.tile([P, H], f16, tag=f"hb{b}k{k}", name=f"hb{b}k{k}")
                        nc.sync.dma_start(t, hid[b][:, k * H : (k + 1) * H])
                        tiles.append(t)
                    hbs[b] = tiles
                else:
                    hb = hpool.tile([P, KT * H], f16, tag=f"hb{b}", name=f"hb{b}")
                    nc.sync.dma_start(hb, hid[b])
                    hbs[b] = hb

            def hb_tile(b, k, m):
                if b < 2:
                    return hbs[b][k][:, m * P : (m + 1) * P]
                return hbs[b][:, k * H + m * P : k * H + (m + 1) * P]

            def load_w(ws, wdram, k, nm):
                ws[k] = wpool.tile([P, H], f16, tag=f"{nm}{k}", name=f"{nm}{k}")
                nc.sync.dma_start(ws[k], wdram[k])

            # DMA order paces the compute pipeline: the first two batches,
            # then w1 (needed by the first fc1 chunk), then batches 2-3, w2,
            # then the tail batches.
            load_mt(0)
            load_hb(0)
            load_mt(1)
            load_hb(1)
            for k in range(KH):
                load_w(w1ks, w1, k, "w1k")
            load_mt(2)
            load_hb(2)
            load_mt(3)
            load_hb(3)
            for k in range(KH):
                load_w(w2ks, w2, k, "w2k")
            for b in range(4, BL):
                load_mt(b)
                load_hb(b)

            xts = [xtpool.tile([P, R], f16, tag=f"xt{k}", name=f"xt{k}") for k in range(KH)]
            y1s = [ypool.tile([P, R], f16, tag=f"y1_{m}", name=f"y1_{m}") for m in range(KH)]
            y2s = [ypool.tile([P, R], f16, tag=f"y2_{m}", name=f"y2_{m}") for m in range(KH)]
            pred = ypool.tile([1, R], f32, tag="pred")

            def pool_b(b):
                mtb = mts[b]
                psums = [ps.tile([P, S], f32, tag="ps", name=f"pp{b}_{m}") for m in range(KH)]
                if b < 2:
                    # k-outer: fire all h-tiles of an arrived t-chunk at once
                    for k in range(KT):
                        for m in range(KH):
                            nc.tensor.matmul(
                                psums[m],
                                lhsT=hb_tile(b, k, m),
                                rhs=mtb[:, k * S : (k + 1) * S],
                                start=(k == 0),
                                stop=(k == KT - 1),
                            )
                else:
                    # m-outer: short psum lifetimes
                    for m in range(KH):
                        for k in range(KT):
                            nc.tensor.matmul(
                                psums[m],
                                lhsT=hb_tile(b, k, m),
                                rhs=mtb[:, k * S : (k + 1) * S],
                                start=(k == 0),
                                stop=(k == KT - 1),
                            )
                # GPSIMD can't read PSUM on TRN2; DVE carries all the copies
                for m in range(KH):
                    nc.vector.tensor_copy(xts[m][:, b * S : (b + 1) * S], psums[m])

            def fc(wks, b_sb, xs, outs, rc, func):
                for m in range(KH):
                    pt = ps.tile([P, RC], f32, tag="ps", name=f"fc{rc}_{m}")
                    for k in range(KH):
                        nc.tensor.matmul(
                            pt,
                            lhsT=wks[k][:, m * P : (m + 1) * P],
                            rhs=xs[k][:, rc * RC : (rc + 1) * RC],
                            start=(k == 0),
                            stop=(k == KH - 1),
                        )
                    nc.scalar.activation(
                        outs[m][:, rc * RC : (rc + 1) * RC],
                        pt,
                        func,
                        bias=b_sb[:, m : m + 1],
                    )

            def fc3(rc):
                pt = ps.tile([1, RC], f32, tag="ps", name=f"fc3_{rc}")
                for k in range(KH):
                    nc.tensor.matmul(
                        pt,
                        lhsT=w3_sb[:, k : k + 1],
                        rhs=y2s[k][:, rc * RC : (rc + 1) * RC],
                        start=(k == 0),
                        stop=(k == KH - 1),
                    )
                nc.scalar.activation(
                    pred[:, rc * RC : (rc + 1) * RC],
                    pt,
                    FT.Sigmoid,
                    bias=b3_sb,
                )
                # stream this chunk's predictions out immediately; only the
                # final 1 KB remains on the critical path after the last sigmoid
                nc.sync.dma_start(
                    out.rearrange("b s -> (b s)")[rc * RC : (rc + 1) * RC],
                    pred[:, rc * RC : (rc + 1) * RC],
                )

            gelu = FT.Gelu
            pool_b(0)
            pool_b(1)
            fc(w1ks, b1_sb, xts, y1s, 0, gelu)
            pool_b(2)
            pool_b(3)
            fc(w1ks, b1_sb, xts, y1s, 1, gelu)
            fc(w2ks, b2_sb, y1s, y2s, 0, gelu)
            fc3(0)
            pool_b(4)
            pool_b(5)
            fc(w1ks, b1_sb, xts, y1s, 2, gelu)
            fc(w2ks, b2_sb, y1s, y2s, 1, gelu)
            fc3(1)
            pool_b(6)
            pool_b(7)
            fc(w1ks, b1_sb, xts, y1s, 3, gelu)
            fc(w2ks, b2_sb, y1s, y2s, 2, gelu)
            fc3(2)
            fc(w2ks, b2_sb, y1s, y2s, 3, gelu)
            fc3(3)

    nc.compile()
    return nc


def _get_program():
    if "nc" not in _CACHE:
        _CACHE["nc"] = _build_program()
    return _CACHE["nc"]


def make_in_maps(hidden, statements_ids, w1, b1, w2, b2, w3, b3):
    hidden = np.asarray(hidden, dtype=np.float32)
    sid = np.asarray(statements_ids, dtype=np.int32)

    # [B, P, KT*H] fp16, partition = token-within-tile
    hid16 = (
        hidden.astype(np.float16)
        .reshape(B, KT, P, H)
        .transpose(0, 2, 1, 3)
        .reshape(B, P, KT * H)
    )
    # count-normalized one-hot: mt[b, t, s] = (sid[b,t]==s) / max(cnt[b,s], 1)
    onehot = sid[:, :, None] == np.arange(S, dtype=np.int32)[None, None, :]
    cnt = onehot.sum(axis=1).astype(np.float32)
    inv = 1.0 / np.maximum(cnt, 1.0)
    mtp = (
        (onehot.astype(np.float32) * inv[:, None, :])
        .astype(np.float16)
        .reshape(B, KT, P, S)
        .transpose(0, 2, 1, 3)
        .reshape(B, P, KT * S)
    )

    w1p = np.ascontiguousarray(
        np.asarray(w1, np.float32).astype(np.float16).reshape(KH, P, H)
    )
    w2p = np.ascontiguousarray(
        np.asarray(w2, np.float32).astype(np.float16).reshape(KH, P, H)
    )
    wpack = np.ascontiguousarray(
        np.asarray(w3, np.float32).astype(np.float16).reshape(KH, P).T
    )
    cpack = np.zeros((P, 13), dtype=np.float32)
    cpack[:, 0:KH] = np.asarray(b1, np.float32).reshape(KH, P).T
    cpack[:, KH : 2 * KH] = np.asarray(b2, np.float32).reshape(KH, P).T
    cpack[0, 12] = np.float32(np.asarray(b3).reshape(-1)[0])

    in_maps = []
    for c in range(N_CORES):
        in_maps.append(
            {
                "hidden": np.ascontiguousarray(hid16[c * BL : (c + 1) * BL]),
                "mtp": np.ascontiguousarray(mtp[c * BL : (c + 1) * BL]),
                "w1": w1p,
                "w2": w2p,
                "wpack": wpack,
                "cpack": cpack,
            }
        )
    return in_maps


def kernel(hidden, statements_ids, w1, b1, w2, b2, w3, b3, **kwargs):
    nc = _get_program()
    in_maps = make_in_maps(hidden, statements_ids, w1, b1, w2, b2, w3, b3)
    trace = bool(int(os.environ.get("KERNEL_TRACE", "0")))
    res = bass_utils.run_bass_kernel_spmd(
        nc, in_maps, core_ids=list(range(N_CORES)), trace=trace
    )
    _CACHE["last_results"] = res
    out = np.concatenate([res.results[c]["out"] for c in range(N_CORES)], axis=0)
    return out.astype(np.float32)
